# revision 1
# baseline (speedup 1.0000x reference)
"""Trainium2 Bass kernel for LocallyConnected1D (filters=1, k=1, no bias):

    out[b, s, 0] = sum_c x[b, s, c] * W[s, c]

x: (256, 8192, 64) f32, W: (8192, 64) f32, out: (256, 8192, 1) f32.

Strategy: pure data-parallel over batch across the 8 NeuronCores (32
batches/core, W replicated per core, no collectives).  Per core, a tile
of 2 batches views as [128 partitions, 8192 free] f32 with W replicated
twice along partitions (pre-replicated on host so it loads with all 128
partitions).

Compute: stock mul+reduce pipelines are DVE-bound above the HBM floor,
so a custom DVE op (registered at build time into the per-NEFF uop
table)

    CUMSUM_MUL:  out[p, t] = sum_{u <= t} in0[p, u] * in1[p, u]

fuses multiply and reduction into ONE 1x pass (~4.3 us per half-tile,
fp32 state; measured 8.69 us for 8192 elems/lane, i.e. ~1.9% over the
1 elem/cycle/lane ideal).  Group sums over the 64 channels are
cumulative-sum differences at the group boundaries t = 64j+63,
extracted by the otherwise-idle GpSimd engine.

Memory: a single HWDGE ring measured ~318 GB/s; both rings together
sustain ~422 GB/s (the SBUF-write fabric ceiling).  Tiles are split
into 1 MiB quarter-chunks alternated across BOTH HWDGE rings (SP via
nc.sync, ACT via nc.scalar) with a 16-slot buffer pool; each chunk gets
its own scan (chunk boundaries are multiples of 64, so a cumsum restart
there stays correct).  W loads as four column quarters ahead of the x
traffic, and output stores ride the SWDGE queue so they never sit
behind a load in a HWDGE ring FIFO.  Measured 190.6 us/core on HW
(baseline 285.6 us): DMA streams continuously at ~422 GB/s with scans
back-to-back underneath.
"""

import sys
from contextlib import ExitStack

import numpy as np

for _p in ("/opt/trn_rl_repo", "/root/.axon_site/_ro/trn_rl_repo"):
    if _p not in sys.path:
        sys.path.insert(0, _p)

import concourse.bacc as bacc
import concourse.mybir as mybir
import concourse.tile as tile
from concourse.bass_utils import run_bass_kernel_spmd

B, S, C = 256, 8192, 64
NCORES = 8
BPC = B // NCORES          # 32 batches per core
BPT = 2                    # batches per tile
NT = BPC // BPT            # 16 tiles per core
P = 128
FREE = BPT * S * C // P    # 8192 elems per partition line
CH = 4                     # DMA/compute chunks per tile
CHW = FREE // CH           # 2048 elems per chunk (1 MiB)
OUT_FREE = BPT * S // P    # 128 outputs per partition line
GCH = CHW // C             # 32 channel groups per chunk

_cache = {}

_CUMMUL_NAME = "CUMSUM_MUL_LL787"


def _register_cummul():
    """Register the fused multiply+cumsum custom DVE op (idempotent).

    body = scan(ADD, Src0 * Src1): an inclusive running sum of the
    elementwise product, fp32 recurrence state, one element/cycle/lane.
    """
    import concourse.dve_ops as dve_ops
    from concourse.dve_spec import AluOp, Spec, Src0, Src1, lower, scan
    from concourse.dve_uop import DveOpSpec

    for o in dve_ops.OPS:
        if o.name == _CUMMUL_NAME:
            return o

    spec = Spec(
        body=scan(AluOp.ADD, Src0 * Src1),
        reference=lambda in0, in1, c0, c1, c2: np.cumsum(
            np.asarray(in0, dtype=np.float32) * np.asarray(in1, dtype=np.float32),
            axis=-1,
            dtype=np.float32,
        ),
    )
    shas = {
        ver: DveOpSpec(
            name=_CUMMUL_NAME, uops=lower(spec, ver=ver), rd1_en=True
        ).sha(ver)
        for ver in ("v3", "v4")
    }
    op = dve_ops.DveOp(_CUMMUL_NAME, spec, subdim=False, uops_sha=shas)
    dve_ops.OPS.append(op)
    dve_ops.CUSTOM_DVE_SPECS[_CUMMUL_NAME] = spec
    dve_ops._SUB_OPCODE_FOR_NAME[_CUMMUL_NAME] = (
        dve_ops._CUSTOM_DVE_ROW_BASE + len(dve_ops.OPS) - 1
    )
    assert dve_ops._SUB_OPCODE_FOR_NAME[_CUMMUL_NAME] < 0x20
    return op


def _build():
    cummul = _register_cummul()

    nc = bacc.Bacc("TRN2", debug=False, target_bir_lowering=False)
    x = nc.dram_tensor("x", [BPC * S * C], mybir.dt.float32, kind="ExternalInput").ap()
    # W pre-replicated to 128 partition rows on the host (2x along batch).
    w = nc.dram_tensor("w", [P * FREE], mybir.dt.float32, kind="ExternalInput").ap()
    out = nc.dram_tensor("out", [BPC * S], mybir.dt.float32, kind="ExternalOutput").ap()

    x_v = x.rearrange("(i p f) -> i p f", i=NT, p=P)      # [16, 128, 8192]
    w_v = w.rearrange("(p f) -> p f", p=P)                # [128, 8192]
    o_v = out.rearrange("(i p j) -> i p j", i=NT, p=P)    # [16, 128, 128]

    with tile.TileContext(nc) as tc, ExitStack() as ctx:
        xp = ctx.enter_context(tc.tile_pool(name="xp", bufs=16))
        wp = ctx.enter_context(tc.tile_pool(name="wp", bufs=1))
        bp = ctx.enter_context(tc.tile_pool(name="bp", bufs=8))

        rings = (nc.sync, nc.scalar)

        # W column quarters, two per HWDGE ring.  W0/W1 lead their rings,
        # but W2/W3 are interleaved AFTER tile 0's first two x chunks:
        # each ring then delivers [W0, x(0,0), W2, x(0,2), ...], so the
        # first scan starts at ~17.5 us instead of waiting out two W
        # transfers (~22.5 us).  Same bytes per ring, same steady state.
        wt = [
            wp.tile([P, CHW], mybir.dt.float32, name=f"wt{c}") for c in range(CH)
        ]
        nc.sync.dma_start(wt[0][:], w_v[:, 0 * CHW : 1 * CHW])
        nc.scalar.dma_start(wt[1][:], w_v[:, 1 * CHW : 2 * CHW])

        for i in range(NT):
            bt = bp.tile([P, OUT_FREE], mybir.dt.float32)
            b3 = bt[:].rearrange("p (j one) -> p j one", one=1)
            for c in range(CH):
                xt = xp.tile([P, CHW], mybir.dt.float32)
                eng = rings[(i * CH + c) % 2]
                eng.dma_start(xt[:], x_v[i][:, c * CHW : (c + 1) * CHW])
                if i == 0 and c == 1:
                    nc.sync.dma_start(wt[2][:], w_v[:, 2 * CHW : 3 * CHW])
                    nc.scalar.dma_start(wt[3][:], w_v[:, 3 * CHW : 4 * CHW])
                # In-place fused multiply + running sum (fp32).
                nc.vector._custom_dve(cummul, out=xt[:], in0=xt[:], in1=wt[c][:])
                # Group sums = cumsum differences at 64-elem boundaries.
                hi = xt[:].rearrange("p (j ch) -> p j ch", ch=C)[:, :, C - 1 : C]
                bh = b3[:, c * GCH : (c + 1) * GCH, :]
                nc.gpsimd.tensor_copy(bh[:, 0:1, :], hi[:, 0:1, :])
                nc.gpsimd.tensor_sub(
                    bh[:, 1:, :], hi[:, 1:, :], hi[:, : GCH - 1, :]
                )
            # Stores ride the SWDGE queue so they never sit behind a 1 MiB
            # load in a HWDGE ring's FIFO (bt recycling gated on them).
            nc.gpsimd.dma_start(o_v[i], bt[:])

    nc.compile()
    return nc


def _get_nc():
    if "nc" not in _cache:
        _cache["nc"] = _build()
    return _cache["nc"]


def run_sharded(x, W, **spmd_kwargs):
    """Shard, run on 8 cores, gather. Returns (out[B, S], BassKernelResults)."""
    nc = _get_nc()
    xf = np.ascontiguousarray(x, dtype=np.float32).reshape(NCORES, BPC * S * C)
    w64 = np.ascontiguousarray(W, dtype=np.float32).reshape(P // 2, FREE)
    wrep = np.concatenate([w64, w64], axis=0).reshape(-1)  # [128*8192]
    in_maps = [{"x": xf[i], "w": wrep} for i in range(NCORES)]
    r = run_bass_kernel_spmd(nc, in_maps, list(range(NCORES)), **spmd_kwargs)
    out = np.concatenate(
        [np.asarray(r.results[i]["out"]).reshape(BPC, S) for i in range(NCORES)],
        axis=0,
    )
    return out, r


def kernel(x, W):
    out, _ = run_sharded(x, W)
    return out[..., None].astype(np.float32)

